# revision 4
# baseline (speedup 1.0000x reference)
"""Quantized Linear (int8-valued GEMM + zero-point corrections) on 8 TRN2 cores.

y = (a @ w).f32 * a_s * w_s
  + (a.f32 * a_s).rowsum * w_o          (per-row correction)
  + a_o * (w.f32 * w_s).colsum          (per-col correction)
  + K * a_o * w_o                       (constant)

Sharding: 2D tensor-parallel grid, 4 shards over M (rows of a) x 2 shards
over N (cols of w).  Each core computes a [1024, 2048] slice of the output.

Device kernel per core (values 0..126 are exact in bf16):
  - main GEMM in bf16 with fp32 PSUM accumulation (exact per-matmul: 128-dot
    of products <= 16129*128 < 2^24)
  - row-sums of a via piggybacked N=1 matmuls sharing the stationary operand
  - col-sums of w via DVE log-halving over k-tiles + an fp32 matmul against a
    beta-filled [128,128] matrix (reduces partitions AND broadcasts in one op)
  - epilogue: out = (psum + beta*colsum_bcast) * (a_s*w_s) + rowbias, where
    rowbias = rowsum * (a_s*w_o) + K*a_o*w_o and beta = a_o/a_s

Input scalars are baked into the program as immediates (compiled per call).
"""

import sys

for _p in ("/opt/trn_rl_repo",):
    if _p not in sys.path:
        sys.path.insert(0, _p)

import numpy as np
import ml_dtypes

BF16 = ml_dtypes.bfloat16

P = 128
M, K, N = 4096, 4096, 4096
GM, GN = 4, 2  # shard grid: 4 over M, 2 over N
MC, NC = M // GM, N // GN  # per-core output slice: 1024 x 2048
CW = 512  # n-chunk width (one PSUM bank)
N_CORES = GM * GN

_cached = {}


def _build_program(ko, mo, nch, cw, s1, c1, c2, beta):
    """Build the single-core Bass/Tile program (SPMD: same program, per-core data)."""
    import concourse.bacc as bacc
    import concourse.mybir as mybir
    import concourse.tile as tile

    f32 = mybir.dt.float32
    bf16 = mybir.dt.bfloat16
    ADD = mybir.AluOpType.add
    MULT = mybir.AluOpType.mult

    mc = mo * P
    ncl = nch * cw

    nc = bacc.Bacc(None, target_bir_lowering=False)
    lhsT_d = nc.dram_tensor("lhsT", [P, ko, mc], bf16, kind="ExternalInput")
    rhs_d = nc.dram_tensor("rhs", [P, ko, ncl], bf16, kind="ExternalInput")
    out_d = nc.dram_tensor("out", [P, mo, ncl], f32, kind="ExternalOutput")

    with tile.TileContext(nc) as tc:
        with (
            tc.tile_pool(name="const", bufs=1) as constp,
            tc.tile_pool(name="lhs", bufs=1) as lhsp,
            tc.tile_pool(name="wpool", bufs=2) as wp,
            tc.tile_pool(name="cs1", bufs=1) as cs1p,
            tc.tile_pool(name="cs2", bufs=1) as cs2p,
            tc.tile_pool(name="colbc", bufs=2) as colbcp,
            tc.tile_pool(name="stage", bufs=4) as stagep,
            tc.tile_pool(name="pmain", bufs=3, space="PSUM") as pmain,
            tc.tile_pool(name="pcol", bufs=2, space="PSUM") as pcol,
            tc.tile_pool(name="prs", bufs=1, space="PSUM") as prs,
        ):
            ones_mov = constp.tile([P, 1], bf16)
            nc.vector.memset(ones_mov[:], 1.0)
            # bw_mat = (a_o*w_s) * ones[128,128]; lhsT of the colsum matmul:
            # (bw_mat.T @ cs)[m, n] = a_o*w_s * sum_p cs[p, n]  (reduce+broadcast)
            bw_mat = constp.tile([P, P], f32)
            nc.vector.memset(bw_mat[:], beta)
            c1_t = constp.tile([P, 1], f32)
            nc.vector.memset(c1_t[:], c1)
            c2_t = constp.tile([P, 1], f32)
            nc.vector.memset(c2_t[:], c2)

            rowbias = constp.tile([P, mo], f32)
            rs_ps = prs.tile([P, mo], f32)

            lhsT_sb = lhsp.tile([P, ko, mc], bf16)
            lchunk = max(1, ko // 8)
            for i in range(0, ko, lchunk):
                nc.sync.dma_start(
                    out=lhsT_sb[:, i : i + lchunk, :], in_=lhsT_d[:, i : i + lchunk, :]
                )

            def load_chunk(ci):
                wt = wp.tile([P, ko, cw], bf16, tag="wchunk", name=f"wt{ci}")
                dchunk = max(1, ko // 4)
                for i in range(0, ko, dchunk):
                    nc.sync.dma_start(
                        out=wt[:, i : i + dchunk, :],
                        in_=rhs_d[:, i : i + dchunk, ci * cw : (ci + 1) * cw],
                    )
                return wt

            def colsum_bcast(ci, wt):
                # reduce over k-tiles: one exact bf16 level (sums <= 252), then f32
                h = ko // 2
                s1t = cs1p.tile([P, h, cw], bf16, tag="cs_bf", name=f"cs1_{ci}")
                nc.vector.tensor_add(s1t[:], wt[:, 0:h, :], wt[:, h : 2 * h, :])
                h //= 2
                s2t = cs2p.tile([P, max(h, 1), cw], f32, tag="cs_f32", name=f"cs2_{ci}")
                if h >= 1:
                    nc.vector.tensor_add(s2t[:, 0:h], s1t[:, 0:h, :], s1t[:, h : 2 * h, :])
                else:
                    nc.vector.tensor_copy(out=s2t[:, 0:1], in_=s1t[:, 0:1, :])
                while h > 1:
                    h //= 2
                    nc.vector.tensor_add(s2t[:, 0:h], s2t[:, 0:h], s2t[:, h : 2 * h])
                # fp32 matmul: partition-reduce + broadcast + beta scale in one shot
                pc = pcol.tile([P, cw], f32, tag="pcol", name=f"pc{ci}")
                nc.tensor.matmul(
                    pc[:], bw_mat[:], s2t[:, 0, :], start=True, stop=True
                )
                col_sb = colbcp.tile([P, cw], f32, tag="colbc", name=f"colsb{ci}")
                nc.scalar.copy(out=col_sb[:], in_=pc[:])
                return col_sb

            wt = load_chunk(0)
            col_sb = colsum_bcast(0, wt)
            for ci in range(nch):
                if ci + 1 < nch:
                    wt_next = load_chunk(ci + 1)
                    col_next = colsum_bcast(ci + 1, wt_next)
                for mi in range(mo):
                    ps = pmain.tile([P, cw], f32, tag="pmain", name=f"ps_{ci}_{mi}")
                    for kt in range(ko):
                        lhs_ap = lhsT_sb[:, kt, mi * P : (mi + 1) * P]
                        nc.tensor.matmul(
                            ps[:],
                            lhs_ap,
                            wt[:, kt, :],
                            start=(kt == 0),
                            stop=(kt == ko - 1),
                        )
                        if ci == 0:
                            # same stationary operand as the main matmul above
                            nc.tensor.matmul(
                                rs_ps[:, mi : mi + 1],
                                lhs_ap,
                                ones_mov[:],
                                start=(kt == 0),
                                stop=(kt == ko - 1),
                            )
                    if ci == 0:
                        # rowbias = rowsum * (a_s*w_o) + K*a_o*w_o
                        nc.vector.tensor_tensor(
                            out=rowbias[:, mi : mi + 1],
                            in0=rs_ps[:, mi : mi + 1],
                            in1=c1_t[:],
                            op=MULT,
                        )
                        nc.vector.tensor_tensor(
                            out=rowbias[:, mi : mi + 1],
                            in0=rowbias[:, mi : mi + 1],
                            in1=c2_t[:],
                            op=ADD,
                        )
                    st = stagep.tile([P, cw], f32, tag="stage", name=f"st_{ci}_{mi}")
                    # st = ps*s1 + rowbias   (scalar engine, per-partition bias)
                    nc.scalar.activation(
                        st[:],
                        ps[:],
                        mybir.ActivationFunctionType.Identity,
                        bias=rowbias[:, mi : mi + 1],
                        scale=s1,
                    )
                    nc.vector.tensor_add(st[:], st[:], col_sb[:])
                    nc.sync.dma_start(
                        out=out_d[:, mi, ci * cw : (ci + 1) * cw], in_=st[:]
                    )
                if ci + 1 < nch:
                    wt = wt_next
                    col_sb = col_next
    nc.compile()
    return nc


def _get_program(ko, mo, nch, cw, s1, c1, c2, beta):
    key = (ko, mo, nch, cw, float(s1), float(c1), float(c2), float(beta))
    if key not in _cached:
        _cached[key] = _build_program(ko, mo, nch, cw, s1, c1, c2, beta)
    return _cached[key]


def _scalars(a_s, a_o, w_s, w_o, k):
    a_s_f = np.float32(np.asarray(a_s).reshape(-1)[0])
    a_o_f = np.float32(np.asarray(a_o).reshape(-1)[0])
    w_s_f = np.float32(np.asarray(w_s).reshape(-1)[0])
    w_o_f = np.float32(np.asarray(w_o).reshape(-1)[0])
    s1 = float(a_s_f * w_s_f)
    c1 = float(a_s_f * w_o_f)
    c2 = float(np.float32(k) * a_o_f * w_o_f)
    bw = float(a_o_f * w_s_f)  # colsum scale (== beta*s1, computed directly)
    return s1, c1, c2, bw


def _make_in_maps(a, w, gm, gn):
    m, k = a.shape
    _, n = w.shape
    mc, ncl = m // gm, n // gn
    ko = k // P

    a_bf = a.astype(BF16)
    w_bf = w.astype(BF16)

    in_maps = []
    for mi in range(gm):
        # aT slice tiled to [P, KO, MC]: [p, kt, mm] = a[mi*mc + mm, kt*128 + p]
        a_sl = a_bf[mi * mc : (mi + 1) * mc, :]  # [mc, k]
        lhsT = np.ascontiguousarray(a_sl.T.reshape(ko, P, mc).transpose(1, 0, 2))
        for nj in range(gn):
            w_sl = w_bf[:, nj * ncl : (nj + 1) * ncl]  # [k, ncl]
            rhs = np.ascontiguousarray(w_sl.reshape(ko, P, ncl).transpose(1, 0, 2))
            in_maps.append({"lhsT": lhsT, "rhs": rhs})
    return in_maps


def _run(a, a_s, a_o, w, w_s, w_o, gm=GM, gn=GN, cw=CW, trace=False):
    from concourse.bass_utils import run_bass_kernel_spmd

    m, k = a.shape
    _, n = w.shape
    mc, ncl = m // gm, n // gn
    s1, c1, c2, beta = _scalars(a_s, a_o, w_s, w_o, k)
    nc = _get_program(k // P, mc // P, ncl // cw, cw, s1, c1, c2, beta)
    in_maps = _make_in_maps(a, w, gm, gn)
    res = run_bass_kernel_spmd(nc, in_maps, list(range(gm * gn)), trace=trace)

    out = np.empty((m, n), dtype=np.float32)
    for mi in range(gm):
        for nj in range(gn):
            r = res.results[mi * gn + nj]["out"]  # [P, MO, NCL]
            out[mi * mc : (mi + 1) * mc, nj * ncl : (nj + 1) * ncl] = (
                r.transpose(1, 0, 2).reshape(mc, ncl)
            )
    return out, res


def kernel(a, a_s, a_o, w, w_s, w_o):
    out, _ = _run(
        np.asarray(a), np.asarray(a_s), np.asarray(a_o),
        np.asarray(w), np.asarray(w_s), np.asarray(w_o),
    )
    return out


# revision 5
# speedup vs baseline: 1.0206x; 1.0206x over previous
"""Quantized Linear (int8-valued GEMM + zero-point corrections) on 8 TRN2 cores.

y = (a @ w).f32 * a_s * w_s
  + (a.f32 * a_s).rowsum * w_o          (per-row correction)
  + a_o * (w.f32 * w_s).colsum          (per-col correction)
  + K * a_o * w_o                       (constant)

Sharding: 2D tensor-parallel grid, 4 shards over M (rows of a) x 2 shards
over N (cols of w).  Each core computes a [1024, 2048] slice of the output.

Device kernel per core (values 0..126 are exact in bf16):
  - main GEMM in bf16 with fp32 PSUM accumulation (exact per-matmul: 128-dot
    of products <= 16129*128 < 2^24)
  - row-sums of a via piggybacked N=1 matmuls sharing the stationary operand
  - col-sums of w via DVE log-halving over k-tiles + an fp32 matmul against a
    beta-filled [128,128] matrix (reduces partitions AND broadcasts in one op)
  - epilogue: out = (psum + beta*colsum_bcast) * (a_s*w_s) + rowbias, where
    rowbias = rowsum * (a_s*w_o) + K*a_o*w_o and beta = a_o/a_s

Input scalars are baked into the program as immediates (compiled per call).
"""

import sys

for _p in ("/opt/trn_rl_repo",):
    if _p not in sys.path:
        sys.path.insert(0, _p)

import numpy as np
import ml_dtypes

BF16 = ml_dtypes.bfloat16

P = 128
M, K, N = 4096, 4096, 4096
GM, GN = 4, 2  # shard grid: 4 over M, 2 over N
MC, NC = M // GM, N // GN  # per-core output slice: 1024 x 2048
CW = 512  # n-chunk width (one PSUM bank)
N_CORES = GM * GN

_cached = {}


def _build_program(ko, mo, nch, cw, s1, c1, c2, beta):
    """Build the single-core Bass/Tile program (SPMD: same program, per-core data)."""
    import concourse.bacc as bacc
    import concourse.mybir as mybir
    import concourse.tile as tile

    f32 = mybir.dt.float32
    bf16 = mybir.dt.bfloat16
    ADD = mybir.AluOpType.add
    MULT = mybir.AluOpType.mult

    mc = mo * P
    ncl = nch * cw

    nc = bacc.Bacc(None, target_bir_lowering=False)
    lhsT_d = nc.dram_tensor("lhsT", [P, mo, ko, P], bf16, kind="ExternalInput")
    rhs_d = nc.dram_tensor("rhs", [P, ko, ncl], bf16, kind="ExternalInput")
    out_d = nc.dram_tensor("out", [P, mo, ncl], f32, kind="ExternalOutput")

    with tile.TileContext(nc) as tc:
        with (
            tc.tile_pool(name="const", bufs=1) as constp,
            tc.tile_pool(name="lhs", bufs=1) as lhsp,
            tc.tile_pool(name="wpool", bufs=2) as wp,
            tc.tile_pool(name="cs1", bufs=1) as cs1p,
            tc.tile_pool(name="cs2", bufs=1) as cs2p,
            tc.tile_pool(name="colbc", bufs=2) as colbcp,
            tc.tile_pool(name="stage", bufs=4) as stagep,
            tc.tile_pool(name="pmain", bufs=3, space="PSUM") as pmain,
            tc.tile_pool(name="pcol", bufs=2, space="PSUM") as pcol,
            tc.tile_pool(name="prs", bufs=1, space="PSUM") as prs,
        ):
            ones_mov = constp.tile([P, 1], bf16)
            nc.vector.memset(ones_mov[:], 1.0)
            # bw_mat = (a_o*w_s) * ones[128,128]; lhsT of the colsum matmul:
            # (bw_mat.T @ cs)[m, n] = a_o*w_s * sum_p cs[p, n]  (reduce+broadcast)
            bw_mat = constp.tile([P, P], f32)
            nc.vector.memset(bw_mat[:], beta)
            c1_t = constp.tile([P, 1], f32)
            nc.vector.memset(c1_t[:], c1)
            c2_t = constp.tile([P, 1], f32)
            nc.vector.memset(c2_t[:], c2)

            rowbias = constp.tile([P, mo], f32)
            rs_ps = prs.tile([P, mo], f32)

            lhsT_sb = lhsp.tile([P, mo, ko, P], bf16)

            def load_lhsT(mi):
                nc.sync.dma_start(
                    out=lhsT_sb[:, mi : mi + 1], in_=lhsT_d[:, mi : mi + 1]
                )

            def load_chunk(ci):
                wt = wp.tile([P, ko, cw], bf16, tag="wchunk", name=f"wt{ci}")
                dchunk = max(1, ko // 4)
                for i in range(0, ko, dchunk):
                    nc.sync.dma_start(
                        out=wt[:, i : i + dchunk, :],
                        in_=rhs_d[:, i : i + dchunk, ci * cw : (ci + 1) * cw],
                    )
                return wt

            def colsum_bcast(ci, wt):
                # reduce over k-tiles: one exact bf16 level (sums <= 252), then f32
                h = ko // 2
                s1t = cs1p.tile([P, h, cw], bf16, tag="cs_bf", name=f"cs1_{ci}")
                nc.vector.tensor_add(s1t[:], wt[:, 0:h, :], wt[:, h : 2 * h, :])
                h //= 2
                s2t = cs2p.tile([P, max(h, 1), cw], f32, tag="cs_f32", name=f"cs2_{ci}")
                if h >= 1:
                    nc.vector.tensor_add(s2t[:, 0:h], s1t[:, 0:h, :], s1t[:, h : 2 * h, :])
                else:
                    nc.vector.tensor_copy(out=s2t[:, 0:1], in_=s1t[:, 0:1, :])
                while h > 1:
                    h //= 2
                    nc.vector.tensor_add(s2t[:, 0:h], s2t[:, 0:h], s2t[:, h : 2 * h])
                # fp32 matmul: partition-reduce + broadcast + beta scale in one shot
                pc = pcol.tile([P, cw], f32, tag="pcol", name=f"pc{ci}")
                nc.tensor.matmul(
                    pc[:], bw_mat[:], s2t[:, 0, :], start=True, stop=True
                )
                col_sb = colbcp.tile([P, cw], f32, tag="colbc", name=f"colsb{ci}")
                nc.scalar.copy(out=col_sb[:], in_=pc[:])
                return col_sb

            wt = load_chunk(0)
            for mi in range(mo):
                load_lhsT(mi)
            col_sb = colsum_bcast(0, wt)
            for ci in range(nch):
                if ci + 1 < nch:
                    wt_next = load_chunk(ci + 1)
                    col_next = colsum_bcast(ci + 1, wt_next)
                for mi in range(mo):
                    ps = pmain.tile([P, cw], f32, tag="pmain", name=f"ps_{ci}_{mi}")
                    for kt in range(ko):
                        lhs_ap = lhsT_sb[:, mi, kt, :]
                        nc.tensor.matmul(
                            ps[:],
                            lhs_ap,
                            wt[:, kt, :],
                            start=(kt == 0),
                            stop=(kt == ko - 1),
                        )
                        if ci == 0:
                            # same stationary operand as the main matmul above
                            nc.tensor.matmul(
                                rs_ps[:, mi : mi + 1],
                                lhs_ap,
                                ones_mov[:],
                                start=(kt == 0),
                                stop=(kt == ko - 1),
                            )
                    if ci == 0:
                        # rowbias = rowsum * (a_s*w_o) + K*a_o*w_o
                        nc.vector.tensor_tensor(
                            out=rowbias[:, mi : mi + 1],
                            in0=rs_ps[:, mi : mi + 1],
                            in1=c1_t[:],
                            op=MULT,
                        )
                        nc.vector.tensor_tensor(
                            out=rowbias[:, mi : mi + 1],
                            in0=rowbias[:, mi : mi + 1],
                            in1=c2_t[:],
                            op=ADD,
                        )
                    st = stagep.tile([P, cw], f32, tag="stage", name=f"st_{ci}_{mi}")
                    # st = ps*s1 + rowbias   (scalar engine, per-partition bias)
                    nc.scalar.activation(
                        st[:],
                        ps[:],
                        mybir.ActivationFunctionType.Identity,
                        bias=rowbias[:, mi : mi + 1],
                        scale=s1,
                    )
                    nc.vector.tensor_add(st[:], st[:], col_sb[:])
                    nc.sync.dma_start(
                        out=out_d[:, mi, ci * cw : (ci + 1) * cw], in_=st[:]
                    )
                if ci + 1 < nch:
                    wt = wt_next
                    col_sb = col_next
    nc.compile()
    return nc


def _get_program(ko, mo, nch, cw, s1, c1, c2, beta):
    key = (ko, mo, nch, cw, float(s1), float(c1), float(c2), float(beta))
    if key not in _cached:
        _cached[key] = _build_program(ko, mo, nch, cw, s1, c1, c2, beta)
    return _cached[key]


def _scalars(a_s, a_o, w_s, w_o, k):
    a_s_f = np.float32(np.asarray(a_s).reshape(-1)[0])
    a_o_f = np.float32(np.asarray(a_o).reshape(-1)[0])
    w_s_f = np.float32(np.asarray(w_s).reshape(-1)[0])
    w_o_f = np.float32(np.asarray(w_o).reshape(-1)[0])
    s1 = float(a_s_f * w_s_f)
    c1 = float(a_s_f * w_o_f)
    c2 = float(np.float32(k) * a_o_f * w_o_f)
    bw = float(a_o_f * w_s_f)  # colsum scale (== beta*s1, computed directly)
    return s1, c1, c2, bw


def _make_in_maps(a, w, gm, gn):
    m, k = a.shape
    _, n = w.shape
    mc, ncl = m // gm, n // gn
    ko = k // P

    a_bf = a.astype(BF16)
    w_bf = w.astype(BF16)

    in_maps = []
    for mi in range(gm):
        # aT slice tiled to [P, KO, MC]: [p, kt, mm] = a[mi*mc + mm, kt*128 + p]
        a_sl = a_bf[mi * mc : (mi + 1) * mc, :]  # [mc, k]
        lhsT = np.ascontiguousarray(
            a_sl.T.reshape(ko, P, mc // P, P).transpose(1, 2, 0, 3)
        )
        for nj in range(gn):
            w_sl = w_bf[:, nj * ncl : (nj + 1) * ncl]  # [k, ncl]
            rhs = np.ascontiguousarray(w_sl.reshape(ko, P, ncl).transpose(1, 0, 2))
            in_maps.append({"lhsT": lhsT, "rhs": rhs})
    return in_maps


def _run(a, a_s, a_o, w, w_s, w_o, gm=GM, gn=GN, cw=CW, trace=False):
    from concourse.bass_utils import run_bass_kernel_spmd

    m, k = a.shape
    _, n = w.shape
    mc, ncl = m // gm, n // gn
    s1, c1, c2, beta = _scalars(a_s, a_o, w_s, w_o, k)
    nc = _get_program(k // P, mc // P, ncl // cw, cw, s1, c1, c2, beta)
    in_maps = _make_in_maps(a, w, gm, gn)
    res = run_bass_kernel_spmd(nc, in_maps, list(range(gm * gn)), trace=trace)

    out = np.empty((m, n), dtype=np.float32)
    for mi in range(gm):
        for nj in range(gn):
            r = res.results[mi * gn + nj]["out"]  # [P, MO, NCL]
            out[mi * mc : (mi + 1) * mc, nj * ncl : (nj + 1) * ncl] = (
                r.transpose(1, 0, 2).reshape(mc, ncl)
            )
    return out, res


def kernel(a, a_s, a_o, w, w_s, w_o):
    out, _ = _run(
        np.asarray(a), np.asarray(a_s), np.asarray(a_o),
        np.asarray(w), np.asarray(w_s), np.asarray(w_o),
    )
    return out
